# revision 6
# baseline (speedup 1.0000x reference)
"""Segment-softmax (GAT stage 4) Trainium2 kernel, 8 NeuronCores.

alpha_i = exp(e_i) / (sum_{j: tgt_j = tgt_i} exp(e_j) + 1e-16)

Design (edge-parallel, bucket-sorted, one-hot width 8):
  - Edges sharded across 8 cores (800k each). Node t factored t = r*8 + q
    with q in [0,8), r in [0,12800) buckets (100 blocks of 128).
  - Host counting-sorts each core's edges by bucket r (pure layout permute)
    into a COMMON column layout (64-bucket groups, per-group uniform chunk
    count = max over cores/buckets) so one SPMD NEFF serves all 8 cores.
    Each 128-edge chunk is r-pure. The host also ships the one-hot encoding
    Q[e, q] = (q_e == q) of the in-chunk node offset (an index-layout
    transform, like the iota/identity constants the device compares against
    otherwise).
  - Phase S (histogram): PE accumulates T[q, r] += Q_chunk^T @ expe_col per
    chunk into PSUM quarters [8, 3200] (start/stop per bucket); rhs is the
    exp(e) column (exp computed on-device). Each chunk costs an 8-column
    LDWEIGHTS + 1-cycle matmul. Quarters are transposed on PE into the
    t-ordered layout Tt[r_lo, blk, q] = T[q, blk*128+r_lo].
  - Phase G: sums the 8 cores' Tt tables on 128 partitions, computes
    W = min(1/(T+1e-16), 6e4) in f16, round-trips it through DRAM, then per
    64-bucket group broadcast-reads the group's [1,512] W slice to all 128
    partitions via DMA. DVE multiplies the one-hot by the broadcast W and
    max-reduces over q via a 3-level pairwise tree (exact: one nonzero per
    row) to get per-edge w = W[t_e]; alpha = expe * w.
"""
import sys

sys.path.insert(0, "/opt/trn_rl_repo")

import numpy as np
import concourse.bacc as bacc
import concourse.mybir as mybir
import concourse.tile as tile
from concourse import bass_utils
from concourse.ap import AP as APC

P = 128
MQ = 8                   # t = r*MQ + q
NB = 12800               # bucket capacity (>= ceil(100000/8)=12500), 100*128
NBLK = NB // P           # 100 transpose blocks
NQRT = 5                 # psum quarters
QRT = NB // NQRT         # 2560 buckets per quarter
GB = 64                  # buckets per group
NGRP = NB // GB          # 200
N_CORES = 8
NUM_EDGES = 6_400_000
NUM_NODES = 100_000
EC = NUM_EDGES // N_CORES

f16, f32 = mybir.dt.float16, mybir.dt.float32
_cache = {}


def _build_s(cnt, C):
    nc = bacc.Bacc("TRN2", target_bir_lowering=False, debug=False,
                   enable_asserts=False)
    d_e = nc.dram_tensor("e", [P, C], f32, kind="ExternalInput")
    d_Q = nc.dram_tensor("Q", [P, C, MQ], f16, kind="ExternalInput")
    d_ident = nc.dram_tensor("ident", [MQ, MQ], f32, kind="ExternalInput")
    d_Tt = nc.dram_tensor("Tt", [P, NBLK, MQ], f32, kind="ExternalOutput")
    d_expe = nc.dram_tensor("expe", [P, C], f16, kind="ExternalOutput")

    # group -> first column
    gcol = np.concatenate([[0], np.cumsum(GB * cnt)]).astype(np.int64)

    with tile.TileContext(nc) as tc:
        with (
            tc.tile_pool(name="const", bufs=1) as cpool,
            tc.tile_pool(name="stage", bufs=1) as spool,
            tc.tile_pool(name="etmp", bufs=2) as epool,
            tc.tile_pool(name="qld", bufs=3) as qpool,
            tc.tile_pool(name="tcp", bufs=2) as tcpool,
            tc.tile_pool(name="psum", bufs=1, space="PSUM") as ppool,
            tc.tile_pool(name="pst", bufs=2, space="PSUM") as ptpool,
        ):
            ident = cpool.tile([MQ, MQ], f32)
            nc.sync.dma_start(out=ident[:], in_=d_ident[:])
            e_sb = spool.tile([P, C], f32)
            expe16 = spool.tile([P, C], f16)
            nc.sync.dma_start(out=e_sb[:], in_=d_e[:])
            STRIP = 2048
            for s0 in range(0, C, STRIP):
                s1 = min(s0 + STRIP, C)
                etmp = epool.tile([P, STRIP], f32, tag="etmp")
                nc.scalar.activation(etmp[:, 0:s1 - s0], e_sb[:, s0:s1],
                                     mybir.ActivationFunctionType.Exp)
                nc.vector.tensor_copy(out=expe16[:, s0:s1],
                                      in_=etmp[:, 0:s1 - s0])

            Tt_sb = spool.tile([P, NBLK, MQ], f32)
            for h in range(NQRT):
                psumT = ppool.tile([MQ, QRT], f32, space="PSUM", tag="ps")
                g0, g1 = h * (NGRP // NQRT), (h + 1) * (NGRP // NQRT)
                for g in range(g0, g1):
                    k = int(cnt[g])
                    ncols = GB * k
                    col = int(gcol[g])
                    Qg = qpool.tile([P, GB * k, MQ], f16, tag="Qg")
                    nc.sync.dma_start(
                        out=Qg[:],
                        in_=d_Q[:, col:col + ncols, :])
                    for b in range(GB):
                        rr = g * GB + b - h * QRT
                        for kk in range(k):
                            c = col + b * k + kk
                            nc.tensor.matmul(out=psumT[:, rr:rr + 1],
                                             lhsT=Qg[:, b * k + kk, :],
                                             rhs=expe16[:, c:c + 1],
                                             start=(kk == 0),
                                             stop=(kk == k - 1))
                # transpose quarter into t-ordered layout
                Tq = tcpool.tile([MQ, QRT], f32, tag="tq")
                nc.vector.tensor_copy(out=Tq[:], in_=psumT[:])
                for b in range(QRT // P):
                    bk = h * (QRT // P) + b
                    pt = ptpool.tile([P, MQ], f32, space="PSUM", tag="pt")
                    nc.tensor.transpose(out=pt[:],
                                        in_=Tq[:, b * P:(b + 1) * P],
                                        identity=ident[:])
                    nc.scalar.copy(out=Tt_sb[:, bk, :], in_=pt[:])
            nc.sync.dma_start(out=d_Tt[:], in_=Tt_sb[:])
            nc.sync.dma_start(out=d_expe[:], in_=expe16[:])
    nc.compile()
    return nc


def _build_g(cnt, C):
    nc = bacc.Bacc("TRN2", target_bir_lowering=False, debug=False,
                   enable_asserts=False)
    d_Ttall = nc.dram_tensor("Ttall", [P, N_CORES, NBLK * MQ], f32,
                             kind="ExternalInput")
    d_Q = nc.dram_tensor("Q", [P, C, MQ], f16, kind="ExternalInput")
    d_expe = nc.dram_tensor("expe", [P, C], f16, kind="ExternalInput")
    d_WT = nc.dram_tensor("WT", [P, NBLK * MQ], f16, kind="ExternalOutput")
    d_alpha = nc.dram_tensor("alpha", [P, C], f32, kind="ExternalOutput")
    OP = mybir.AluOpType

    gcol = np.concatenate([[0], np.cumsum(GB * cnt)]).astype(np.int64)

    with tile.TileContext(nc) as tc:
        with (
            tc.tile_pool(name="stage", bufs=1) as spool,
            tc.tile_pool(name="ttmp", bufs=2) as tpool,
            tc.tile_pool(name="wbc", bufs=3) as wbpool,
            tc.tile_pool(name="qld", bufs=3) as qpool,
            tc.tile_pool(name="work", bufs=2) as wpool,
            tc.tile_pool(name="alph", bufs=2) as apool,
        ):
            expe16 = spool.tile([P, C], f16)
            nc.sync.dma_start(out=expe16[:], in_=d_expe[:])

            # 8-way table all-reduce on 128 partitions
            FT = NBLK * MQ
            Tacc = spool.tile([P, FT], f32)
            nc.sync.dma_start(out=Tacc[:], in_=d_Ttall[:, 0, :])
            for c in range(1, N_CORES):
                tmp = tpool.tile([P, FT], f32, tag="tt")
                nc.sync.dma_start(out=tmp[:], in_=d_Ttall[:, c, :])
                nc.vector.tensor_tensor(out=Tacc[:], in0=Tacc[:],
                                        in1=tmp[:], op=OP.add)
            nc.vector.tensor_scalar_add(out=Tacc[:], in0=Tacc[:],
                                        scalar1=1e-16)
            nc.vector.reciprocal(out=Tacc[:], in_=Tacc[:])
            W16 = spool.tile([P, FT], f16)
            nc.vector.tensor_scalar_min(out=W16[:], in0=Tacc[:],
                                        scalar1=60000.0)
            nc.sync.dma_start(out=d_WT[:], in_=W16[:])

            for g in range(NGRP):
                k = int(cnt[g])
                ncols = GB * k
                col = int(gcol[g])
                # broadcast-read this group's 512 W values to all partitions
                Wbc = wbpool.tile([P, GB, MQ], f16, tag="wbc")
                off0 = (GB * (g % 2)) * FT + (g // 2) * MQ
                src = APC(d_WT[:].tensor, off0, [[0, P], [FT, GB], [1, MQ]])
                nc.sync.dma_start(out=Wbc[:], in_=src)

                Qg = qpool.tile([P, GB * k, MQ], f16, tag="Qg")
                nc.sync.dma_start(out=Qg[:], in_=d_Q[:, col:col + ncols, :])
                Pg = wpool.tile([P, GB * k, MQ], f16, tag="Pg")
                nc.vector.tensor_tensor(
                    out=Pg[:].rearrange("p (b k) q -> p b k q", b=GB),
                    in0=Qg[:].rearrange("p (b k) q -> p b k q", b=GB),
                    in1=Wbc[:][:, :, None, :].broadcast_to([P, GB, k, MQ]),
                    op=OP.mult)
                # 3-level pairwise max tree over q (exact: one nonzero)
                m1 = wpool.tile([P, GB * k, 4], f16, tag="m1")
                nc.vector.tensor_tensor(out=m1[:], in0=Pg[:, :, 0:4],
                                        in1=Pg[:, :, 4:8], op=OP.max)
                m2 = wpool.tile([P, GB * k, 2], f16, tag="m2")
                nc.vector.tensor_tensor(out=m2[:], in0=m1[:, :, 0:2],
                                        in1=m1[:, :, 2:4], op=OP.max)
                wg = wpool.tile([P, GB * k], f16, tag="wg")
                nc.vector.tensor_tensor(
                    out=wg[:],
                    in0=m2[:, :, 0],
                    in1=m2[:, :, 1],
                    op=OP.max)
                alph = apool.tile([P, GB * k], f32, tag="al")
                nc.vector.tensor_tensor(out=alph[:, 0:ncols],
                                        in0=expe16[:, col:col + ncols],
                                        in1=wg[:, 0:ncols], op=OP.mult)
                nc.sync.dma_start(out=d_alpha[:, col:col + ncols],
                                  in_=alph[:, 0:ncols])
    nc.compile()
    return nc


def _get_neffs(meta):
    key = (meta["C"], meta["cnt"].tobytes())
    if key not in _cache:
        _cache[key] = (_build_s(meta["cnt"], meta["C"]),
                       _build_g(meta["cnt"], meta["C"]))
    return _cache[key]


def prep_inputs(e, edge_index):
    e = np.asarray(e, dtype=np.float32).reshape(-1)
    t = np.asarray(edge_index)[1].astype(np.int64)
    r_all = (t // MQ).astype(np.int32).reshape(N_CORES, EC)
    q_all = (t % MQ).astype(np.int32).reshape(N_CORES, EC)
    e_all = e.reshape(N_CORES, EC)

    counts = np.empty((N_CORES, NB), np.int64)
    for c in range(N_CORES):
        counts[c] = np.bincount(r_all[c], minlength=NB)
    need = -(-counts.max(axis=0) // P)          # ceil
    cnt = np.maximum(need.reshape(NGRP, GB).max(axis=1), 1).astype(np.int32)
    C = int((GB * cnt).sum())
    gbase = np.concatenate([[0], np.cumsum(GB * cnt)])[:-1]
    base = np.repeat(gbase, GB) + (np.arange(NB) % GB) * np.repeat(cnt, GB)

    ident = np.eye(MQ, dtype=np.float32)

    in_maps_s, placements = [], []
    for c in range(N_CORES):
        r, q, ec = r_all[c], q_all[c], e_all[c]
        order = np.argsort(r, kind="stable")
        rs = r[order]
        starts = np.concatenate([[0], np.cumsum(counts[c])])
        rank = np.arange(EC, dtype=np.int64) - starts[rs]
        part = (rank % P).astype(np.int32)
        colp = (base[rs] + rank // P).astype(np.int32)
        e_pad = np.full((P, C), -100.0, np.float32)
        e_pad[part, colp] = ec[order]
        Q = np.zeros((P, C, MQ), np.float16)
        Q[part, colp, q[order]] = 1.0
        in_maps_s.append({"e": e_pad, "Q": Q, "ident": ident})
        placements.append((order, part, colp))

    meta = {"C": C, "cnt": cnt, "placements": placements}
    return in_maps_s, meta


def make_g_maps(res_s, meta):
    Ttall = np.stack([res_s.results[c]["Tt"].reshape(P, NBLK * MQ)
                      for c in range(N_CORES)], axis=1)
    return [{"Ttall": Ttall, "Q": None, "expe": res_s.results[c]["expe"]}
            for c in range(N_CORES)]


def unpack_alpha(res_g, meta):
    alpha = np.empty(NUM_EDGES, dtype=np.float32)
    for c in range(N_CORES):
        order, part, colp = meta["placements"][c]
        a = res_g.results[c]["alpha"]
        shard = np.empty(EC, dtype=np.float32)
        shard[order] = a[part, colp]
        alpha[c * EC:(c + 1) * EC] = shard
    return alpha


def kernel(e, edge_index, num_nodes):
    assert int(num_nodes) == NUM_NODES
    in_maps_s, meta = prep_inputs(e, edge_index)
    nc_s, nc_g = _get_neffs(meta)
    res_s = bass_utils.run_bass_kernel_spmd(nc_s, in_maps_s,
                                            core_ids=list(range(N_CORES)))
    in_maps_g = make_g_maps(res_s, meta)
    for c in range(N_CORES):
        in_maps_g[c]["Q"] = in_maps_s[c]["Q"]
    res_g = bass_utils.run_bass_kernel_spmd(nc_g, in_maps_g,
                                            core_ids=list(range(N_CORES)))
    return unpack_alpha(res_g, meta)


# revision 12
# speedup vs baseline: 2.9859x; 2.9859x over previous
"""Segment-softmax (GAT stage 4) Trainium2 kernel, 8 NeuronCores.

alpha_i = exp(e_i) / (sum_{j: tgt_j = tgt_i} exp(e_j) + 1e-16)

Two phases, two different host-side bucket-sorted layouts (pure permutes):

  Phase S (histogram), MQ=32: node t = r*32 + q. Edges bucket-sorted by r
  (3200 buckets, per-bucket chunk count = max over the 8 cores so one SPMD
  NEFF fits all). Per 128-edge r-pure chunk, PE accumulates
  T[q, r] += onehot_q^T @ expe_col into PSUM (start/stop per bucket);
  the one-hot is built on-device (DVE/GPSIMD is_equal split). The table is
  transposed on PE and written to DRAM in flat t-order (factorization-
  independent).

  Phase G (gather/divide), MQ=8: its own bucket sort by t//8 (12800
  buckets, mean 62.5 edges -> 1 chunk per bucket). The 8 cores' flat
  tables are summed on 128 partitions, W = min(1/(T+1e-16), 6e4) in f16
  is round-tripped through DRAM, and each 512-bucket group's W slice is a
  CONTIGUOUS flat range broadcast to all partitions as one fat DMA.
  The host ships the 8-wide one-hot Q8 (an index-encoding layout); DVE
  multiplies Q8 by the broadcast W and max-reduces over q via a 3-level
  pairwise tree (exact - one nonzero per row), then alpha = expe * w with
  exp(e) recomputed on-device in the G layout.
"""
import sys

sys.path.insert(0, "/opt/trn_rl_repo")

import numpy as np
import concourse.bacc as bacc
import concourse.mybir as mybir
import concourse.tile as tile
from concourse import bass_utils
from concourse.ap import AP as APC

P = 128
NUM_EDGES = 6_400_000
NUM_NODES = 100_000
N_CORES = 8
EC = NUM_EDGES // N_CORES
NT = 102_400             # padded node capacity (t-order flat table size)

# Phase S factorization
MQS = 32
NBS = NT // MQS          # 3200 buckets
NHLF = 5                 # psum sections (640 buckets = 5x128 blocks each)
HLF = NBS // NHLF        # 640 buckets per section

# Phase G factorization
MQG = 8
NBG = NT // MQG          # 12800 buckets
GBG = 512                # buckets per G group
NGG = NBG // GBG         # 25 groups

f16, f32 = mybir.dt.float16, mybir.dt.float32
_cache = {}


def _build_s(cntS, CS):
    nc = bacc.Bacc("TRN2", target_bir_lowering=False, debug=False,
                   enable_asserts=False)
    d_e = nc.dram_tensor("e", [P, CS], f32, kind="ExternalInput")
    d_q = nc.dram_tensor("q", [P, CS], f16, kind="ExternalInput")
    d_iota = nc.dram_tensor("iota", [P, MQS], f16, kind="ExternalInput")
    d_ident = nc.dram_tensor("ident", [MQS, MQS], f32, kind="ExternalInput")
    d_T = nc.dram_tensor("T", [1, NT], f32, kind="ExternalOutput")
    OP = mybir.AluOpType

    base = np.concatenate([[0], np.cumsum(cntS)]).astype(np.int64)

    with tile.TileContext(nc) as tc:
        with (
            tc.tile_pool(name="const", bufs=1) as cpool,
            tc.tile_pool(name="stage", bufs=1) as spool,
            tc.tile_pool(name="etmp", bufs=2) as epool,
            tc.tile_pool(name="work", bufs=3) as wpool,
            tc.tile_pool(name="psum", bufs=2, space="PSUM") as ppool,
            tc.tile_pool(name="pst", bufs=2, space="PSUM") as ptpool,
        ):
            iq = cpool.tile([P, MQS], f16)
            ident = cpool.tile([MQS, MQS], f32)
            nc.sync.dma_start(out=iq[:], in_=d_iota[:])
            nc.sync.dma_start(out=ident[:], in_=d_ident[:])
            q_sb = spool.tile([P, CS], f16)
            e_sb = spool.tile([P, CS], f32)
            expe16 = spool.tile([P, CS], f16)
            nc.sync.dma_start(out=q_sb[:], in_=d_q[:])
            nc.sync.dma_start(out=e_sb[:], in_=d_e[:])
            STRIP = 2048
            for s0 in range(0, CS, STRIP):
                s1 = min(s0 + STRIP, CS)
                etmp = epool.tile([P, STRIP], f32, tag="etmp")
                nc.scalar.activation(etmp[:, 0:s1 - s0], e_sb[:, s0:s1],
                                     mybir.ActivationFunctionType.Exp)
                nc.vector.tensor_copy(out=expe16[:, s0:s1],
                                      in_=etmp[:, 0:s1 - s0])

            # batched one-hot build: 256-column windows built lazily just
            # before their consuming matmuls; DVE/GPSIMD is_equal split
            QB = 256
            win_tiles = {}

            def get_win(w):
                if w not in win_tiles:
                    c0 = w * QB
                    c1 = min(c0 + QB, CS)
                    ncols = c1 - c0
                    Qw = wpool.tile([P, QB, MQS], f16, tag="Qw")
                    nc.vector.tensor_tensor(
                        out=Qw[:, 0:ncols, :],
                        in0=q_sb[:, c0:c1][:, :, None].broadcast_to(
                            [P, ncols, MQS]),
                        in1=iq[:][:, None, :].broadcast_to([P, ncols, MQS]),
                        op=OP.is_equal)
                    win_tiles[w] = Qw
                return win_tiles[w]

            for h in range(NHLF):
                psumT = ppool.tile([MQS, HLF], f32, space="PSUM", tag="ps")
                for rr in range(h * HLF, (h + 1) * HLF):
                    k = int(cntS[rr])
                    for kk in range(k):
                        c = int(base[rr]) + kk
                        Qw = get_win(c // QB)
                        nc.tensor.matmul(out=psumT[:, rr - h * HLF:
                                                   rr - h * HLF + 1],
                                         lhsT=Qw[:, c % QB, :],
                                         rhs=expe16[:, c:c + 1],
                                         start=(kk == 0), stop=(kk == k - 1))
                Tq = spool.tile([MQS, HLF], f32, tag=f"tq{h}")
                nc.vector.tensor_copy(out=Tq[:], in_=psumT[:])
                for b in range(HLF // P):
                    bk = h * (HLF // P) + b
                    pt = ptpool.tile([P, MQS], f32, space="PSUM", tag="pt")
                    nc.tensor.transpose(out=pt[:],
                                        in_=Tq[:, b * P:(b + 1) * P],
                                        identity=ident[:])
                    ptc = epool.tile([P, MQS], f32, tag="ptc")
                    nc.scalar.copy(out=ptc[:], in_=pt[:])
                    # flat t-order: t = (bk*128 + r_lo)*32 + q
                    dst = APC(d_T[:].tensor, bk * P * MQS,
                              [[MQS, P], [1, MQS]])
                    nc.sync.dma_start(out=dst, in_=ptc[:])
    nc.compile()
    return nc


def _build_g(cntG, CG):
    nc = bacc.Bacc("TRN2", target_bir_lowering=False, debug=False,
                   enable_asserts=False)
    d_Tall = nc.dram_tensor("Tall", [P, N_CORES, NT // P], f32,
                            kind="ExternalInput")
    d_e = nc.dram_tensor("e", [P, CG], f32, kind="ExternalInput")
    d_Q = nc.dram_tensor("Q", [P, CG, MQG], f16, kind="ExternalInput")
    d_WT = nc.dram_tensor("WT", [P, NT // P], f16, kind="ExternalOutput")
    d_alpha = nc.dram_tensor("alpha", [P, CG], f32, kind="ExternalOutput")
    OP = mybir.AluOpType

    gcol = np.concatenate([[0], np.cumsum(GBG * cntG)]).astype(np.int64)

    with tile.TileContext(nc) as tc:
        with (
            tc.tile_pool(name="stage", bufs=1) as spool,
            tc.tile_pool(name="ttmp", bufs=2) as tpool,
            tc.tile_pool(name="etmp", bufs=2) as epool,
            tc.tile_pool(name="wbc", bufs=2) as wbpool,
            tc.tile_pool(name="qld", bufs=2) as qpool,
            tc.tile_pool(name="work", bufs=2) as wpool,
            tc.tile_pool(name="alph", bufs=2) as apool,
        ):
            # exp(e) in G layout
            e_sb = spool.tile([P, CG], f32)
            expe16 = spool.tile([P, CG], f16)
            nc.sync.dma_start(out=e_sb[:], in_=d_e[:])
            STRIP = 2048
            for s0 in range(0, CG, STRIP):
                s1 = min(s0 + STRIP, CG)
                etmp = epool.tile([P, STRIP], f32, tag="etmp")
                nc.scalar.activation(etmp[:, 0:s1 - s0], e_sb[:, s0:s1],
                                     mybir.ActivationFunctionType.Exp)
                nc.vector.tensor_copy(out=expe16[:, s0:s1],
                                      in_=etmp[:, 0:s1 - s0])

            # 8-way flat-table all-reduce on 128 partitions
            FT = NT // P
            Tacc = spool.tile([P, FT], f32)
            nc.sync.dma_start(out=Tacc[:], in_=d_Tall[:, 0, :])
            for c in range(1, N_CORES):
                tmp = tpool.tile([P, FT], f32, tag="tt")
                nc.sync.dma_start(out=tmp[:], in_=d_Tall[:, c, :])
                nc.vector.tensor_tensor(out=Tacc[:], in0=Tacc[:],
                                        in1=tmp[:], op=OP.add)
            nc.vector.tensor_scalar_add(out=Tacc[:], in0=Tacc[:],
                                        scalar1=1e-16)
            nc.vector.reciprocal(out=Tacc[:], in_=Tacc[:])
            W16 = spool.tile([P, FT], f16)
            nc.vector.tensor_scalar_min(out=W16[:], in0=Tacc[:],
                                        scalar1=60000.0)
            nc.sync.dma_start(out=d_WT[:], in_=W16[:])

            for g in range(NGG):
                k = int(cntG[g])
                ncols = GBG * k
                col = int(gcol[g])
                # this group's W slice: flat t range [g*4096, (g+1)*4096)
                # broadcast to all partitions - contiguous per-partition run
                Wbc = wbpool.tile([P, GBG * MQG], f16, tag="wbc")
                src = APC(d_WT[:].tensor, g * GBG * MQG,
                          [[0, P], [1, GBG * MQG]])
                nc.sync.dma_start(out=Wbc[:], in_=src)

                Qg = qpool.tile([P, GBG * k, MQG], f16, tag="Qg")
                nc.sync.dma_start(out=Qg[:], in_=d_Q[:, col:col + ncols, :])
                Pg = wpool.tile([P, GBG * k, MQG], f16, tag="Pg")
                nc.vector.tensor_tensor(
                    out=Pg[:].rearrange("p (b k) q -> p b k q", b=GBG),
                    in0=Qg[:].rearrange("p (b k) q -> p b k q", b=GBG),
                    in1=Wbc[:].rearrange("p (b q) -> p b q", b=GBG)
                    [:, :, None, :].broadcast_to([P, GBG, k, MQG]),
                    op=OP.mult)
                # 3-level pairwise max tree over q (exact: one nonzero)
                m1 = wpool.tile([P, GBG * k, 4], f16, tag="m1")
                nc.vector.tensor_tensor(out=m1[:], in0=Pg[:, :, 0:4],
                                        in1=Pg[:, :, 4:8], op=OP.max)
                m2 = wpool.tile([P, GBG * k, 2], f16, tag="m2")
                nc.vector.tensor_tensor(out=m2[:], in0=m1[:, :, 0:2],
                                        in1=m1[:, :, 2:4], op=OP.max)
                wg = wpool.tile([P, GBG * k], f16, tag="wg")
                nc.vector.tensor_tensor(out=wg[:], in0=m2[:, :, 0],
                                        in1=m2[:, :, 1], op=OP.max)
                alph = apool.tile([P, GBG * k], f32, tag="al")
                nc.vector.tensor_tensor(out=alph[:, 0:ncols],
                                        in0=expe16[:, col:col + ncols],
                                        in1=wg[:, 0:ncols], op=OP.mult)
                nc.sync.dma_start(out=d_alpha[:, col:col + ncols],
                                  in_=alph[:, 0:ncols])
    nc.compile()
    return nc


def _get_neffs(meta):
    key = (meta["CS"], meta["cntS"].tobytes(),
           meta["CG"], meta["cntG"].tobytes())
    if key not in _cache:
        _cache[key] = (_build_s(meta["cntS"], meta["CS"]),
                       _build_g(meta["cntG"], meta["CG"]))
    return _cache[key]


def _layout(t, mq, nb, group):
    """Common bucket-sorted layout. Returns per-group cnt, per-bucket
    capacities, total columns C, and per-core (order, part, col)."""
    r_all = (t // mq).astype(np.int32).reshape(N_CORES, EC)
    counts = np.empty((N_CORES, nb), np.int64)
    for c in range(N_CORES):
        counts[c] = np.bincount(r_all[c], minlength=nb)
    need = -(-counts.max(axis=0) // P)            # ceil over max core count
    ngrp = nb // group
    cnt = np.maximum(need.reshape(ngrp, group).max(axis=1), 1).astype(
        np.int32)
    percnt = np.repeat(cnt, group)                # per-bucket capacity
    C = int(percnt.sum())
    base = np.concatenate([[0], np.cumsum(percnt)])[:-1]
    placements = []
    for c in range(N_CORES):
        r = r_all[c]
        order = np.argsort(r, kind="stable")
        rs = r[order]
        starts = np.concatenate([[0], np.cumsum(counts[c])])
        rank = np.arange(EC, dtype=np.int64) - starts[rs]
        part = (rank % P).astype(np.int32)
        col = (base[rs] + rank // P).astype(np.int32)
        placements.append((order, part, col))
    return cnt, percnt, C, placements


def prep_inputs(e, edge_index):
    e = np.asarray(e, dtype=np.float32).reshape(-1)
    t = np.asarray(edge_index)[1].astype(np.int64)
    e_all = e.reshape(N_CORES, EC)
    qS_all = (t % MQS).astype(np.int32).reshape(N_CORES, EC)
    qG_all = (t % MQG).astype(np.int32).reshape(N_CORES, EC)

    cntS_g, percntS, CS, placeS = _layout(t, MQS, NBS, 1)
    cntG, percntG, CG, placeG = _layout(t, MQG, NBG, GBG)

    iota = np.arange(MQS, dtype=np.float16)[None, :].repeat(P, axis=0)
    ident = np.eye(MQS, dtype=np.float32)

    in_maps_s, in_maps_g = [], []
    for c in range(N_CORES):
        orderS, partS, colS = placeS[c]
        e_padS = np.full((P, CS), -100.0, np.float32)
        q16 = np.zeros((P, CS), np.float16)
        e_padS[partS, colS] = e_all[c][orderS]
        q16[partS, colS] = qS_all[c][orderS].astype(np.float16)
        in_maps_s.append({"e": e_padS, "q": q16, "iota": iota,
                          "ident": ident})

        orderG, partG, colG = placeG[c]
        e_padG = np.full((P, CG), -100.0, np.float32)
        Q8 = np.zeros((P, CG, MQG), np.float16)
        e_padG[partG, colG] = e_all[c][orderG]
        Q8[partG, colG, qG_all[c][orderG]] = 1.0
        in_maps_g.append({"e": e_padG, "Q": Q8, "Tall": None})

    meta = {"CS": CS, "cntS": percntS.astype(np.int32),
            "CG": CG, "cntG": cntG, "placeG": placeG,
            "in_maps_g": in_maps_g}
    return in_maps_s, meta


def make_g_maps(res_s, meta):
    Tall = np.stack([res_s.results[c]["T"].reshape(P, NT // P)
                     for c in range(N_CORES)], axis=1)
    for c in range(N_CORES):
        meta["in_maps_g"][c]["Tall"] = Tall
    return meta["in_maps_g"]


def unpack_alpha(res_g, meta):
    alpha = np.empty(NUM_EDGES, dtype=np.float32)
    for c in range(N_CORES):
        order, part, col = meta["placeG"][c]
        a = res_g.results[c]["alpha"]
        shard = np.empty(EC, dtype=np.float32)
        shard[order] = a[part, col]
        alpha[c * EC:(c + 1) * EC] = shard
    return alpha


def kernel(e, edge_index, num_nodes):
    assert int(num_nodes) == NUM_NODES
    in_maps_s, meta = prep_inputs(e, edge_index)
    nc_s, nc_g = _get_neffs(meta)
    res_s = bass_utils.run_bass_kernel_spmd(nc_s, in_maps_s,
                                            core_ids=list(range(N_CORES)))
    in_maps_g = make_g_maps(res_s, meta)
    res_g = bass_utils.run_bass_kernel_spmd(nc_g, in_maps_g,
                                            core_ids=list(range(N_CORES)))
    return unpack_alpha(res_g, meta)


# revision 14
# speedup vs baseline: 3.0336x; 1.0160x over previous
"""Segment-softmax (GAT stage 4) Trainium2 kernel, 8 NeuronCores.

alpha_i = exp(e_i) / (sum_{j: tgt_j = tgt_i} exp(e_j) + 1e-16)

Two phases, two different host-side bucket-sorted layouts (pure permutes):

  Phase S (histogram), MQ=32: node t = r*32 + q. Edges bucket-sorted by r
  (3200 buckets, per-bucket chunk count = max over the 8 cores so one SPMD
  NEFF fits all). Per 128-edge r-pure chunk, PE accumulates
  T[q, r] += onehot_q^T @ expe_col into PSUM (start/stop per bucket);
  the one-hot is built on-device (DVE/GPSIMD is_equal split). The table is
  transposed on PE and written to DRAM in flat t-order (factorization-
  independent).

  Phase G (gather/divide), MQ=8: its own bucket sort by t//8 (12800
  buckets, mean 62.5 edges -> 1 chunk per bucket). The 8 cores' flat
  tables are summed on 128 partitions, W = min(1/(T+1e-16), 6e4) in f16
  is round-tripped through DRAM, and each 512-bucket group's W slice is a
  CONTIGUOUS flat range broadcast to all partitions as one fat DMA.
  The host ships the 8-wide one-hot Q8 (an index-encoding layout); DVE
  multiplies Q8 by the broadcast W and max-reduces over q via a 3-level
  pairwise tree (exact - one nonzero per row), then alpha = expe * w with
  exp(e) recomputed on-device in the G layout.
"""
import sys

sys.path.insert(0, "/opt/trn_rl_repo")

import numpy as np
import concourse.bacc as bacc
import concourse.mybir as mybir
import concourse.tile as tile
from concourse import bass_utils
from concourse.ap import AP as APC

P = 128
NUM_EDGES = 6_400_000
NUM_NODES = 100_000
N_CORES = 8
EC = NUM_EDGES // N_CORES
NT = 102_400             # padded node capacity (t-order flat table size)

# Phase S factorization
MQS = 32
NBS = NT // MQS          # 3200 buckets
NHLF = 5                 # psum sections (640 buckets = 5x128 blocks each)
HLF = NBS // NHLF        # 640 buckets per section

# Phase G factorization
MQG = 8
NBG = NT // MQG          # 12800 buckets
GBG = 512                # buckets per G group
NGG = NBG // GBG         # 25 groups

f16, f32 = mybir.dt.float16, mybir.dt.float32
_cache = {}


def _build_s(cntS, CS):
    nc = bacc.Bacc("TRN2", target_bir_lowering=False, debug=False,
                   enable_asserts=False)
    d_e = nc.dram_tensor("e", [P, CS], f32, kind="ExternalInput")
    d_q = nc.dram_tensor("q", [P, CS], f16, kind="ExternalInput")
    d_iota = nc.dram_tensor("iota", [P, MQS], f16, kind="ExternalInput")
    d_ident = nc.dram_tensor("ident", [MQS, MQS], f32, kind="ExternalInput")
    d_T = nc.dram_tensor("T", [1, NT], f32, kind="ExternalOutput")
    OP = mybir.AluOpType

    base = np.concatenate([[0], np.cumsum(cntS)]).astype(np.int64)

    with tile.TileContext(nc) as tc:
        with (
            tc.tile_pool(name="const", bufs=1) as cpool,
            tc.tile_pool(name="stage", bufs=1) as spool,
            tc.tile_pool(name="etmp", bufs=2) as epool,
            tc.tile_pool(name="work", bufs=3) as wpool,
            tc.tile_pool(name="psum", bufs=2, space="PSUM") as ppool,
            tc.tile_pool(name="pst", bufs=2, space="PSUM") as ptpool,
        ):
            iq = cpool.tile([P, MQS], f16)
            ident = cpool.tile([MQS, MQS], f32)
            nc.sync.dma_start(out=iq[:], in_=d_iota[:])
            nc.sync.dma_start(out=ident[:], in_=d_ident[:])
            q_sb = spool.tile([P, CS], f16)
            e_sb = spool.tile([P, CS], f32)
            expe16 = spool.tile([P, CS], f16)
            nc.sync.dma_start(out=q_sb[:], in_=d_q[:])
            nc.sync.dma_start(out=e_sb[:], in_=d_e[:])
            STRIP = 2048
            for s0 in range(0, CS, STRIP):
                s1 = min(s0 + STRIP, CS)
                nc.scalar.activation(expe16[:, s0:s1], e_sb[:, s0:s1],
                                     mybir.ActivationFunctionType.Exp)

            # batched one-hot build: 256-column windows built lazily just
            # before their consuming matmuls; DVE/GPSIMD is_equal split
            QB = 256
            win_tiles = {}

            def get_win(w):
                if w not in win_tiles:
                    c0 = w * QB
                    c1 = min(c0 + QB, CS)
                    ncols = c1 - c0
                    Qw = wpool.tile([P, QB, MQS], f16, tag="Qw")
                    nc.vector.tensor_tensor(
                        out=Qw[:, 0:ncols, :],
                        in0=q_sb[:, c0:c1][:, :, None].broadcast_to(
                            [P, ncols, MQS]),
                        in1=iq[:][:, None, :].broadcast_to([P, ncols, MQS]),
                        op=OP.is_equal)
                    win_tiles[w] = Qw
                return win_tiles[w]

            for h in range(NHLF):
                psumT = ppool.tile([MQS, HLF], f32, space="PSUM", tag="ps")
                for rr in range(h * HLF, (h + 1) * HLF):
                    k = int(cntS[rr])
                    for kk in range(k):
                        c = int(base[rr]) + kk
                        Qw = get_win(c // QB)
                        nc.tensor.matmul(out=psumT[:, rr - h * HLF:
                                                   rr - h * HLF + 1],
                                         lhsT=Qw[:, c % QB, :],
                                         rhs=expe16[:, c:c + 1],
                                         start=(kk == 0), stop=(kk == k - 1))
                Tq = spool.tile([MQS, HLF], f32, tag=f"tq{h}")
                nc.vector.tensor_copy(out=Tq[:], in_=psumT[:])
                for b in range(HLF // P):
                    bk = h * (HLF // P) + b
                    pt = ptpool.tile([P, MQS], f32, space="PSUM", tag="pt")
                    nc.tensor.transpose(out=pt[:],
                                        in_=Tq[:, b * P:(b + 1) * P],
                                        identity=ident[:])
                    ptc = epool.tile([P, MQS], f32, tag="ptc")
                    nc.scalar.copy(out=ptc[:], in_=pt[:])
                    # flat t-order: t = (bk*128 + r_lo)*32 + q
                    dst = APC(d_T[:].tensor, bk * P * MQS,
                              [[MQS, P], [1, MQS]])
                    nc.sync.dma_start(out=dst, in_=ptc[:])
    nc.compile()
    return nc


def _build_g(cntG, CG):
    nc = bacc.Bacc("TRN2", target_bir_lowering=False, debug=False,
                   enable_asserts=False)
    d_Tall = nc.dram_tensor("Tall", [P, N_CORES, NT // P], f32,
                            kind="ExternalInput")
    d_e = nc.dram_tensor("e", [P, CG], f32, kind="ExternalInput")
    d_Q = nc.dram_tensor("Q", [P, CG, MQG], f16, kind="ExternalInput")
    d_WT = nc.dram_tensor("WT", [P, NT // P], f16, kind="ExternalOutput")
    d_alpha = nc.dram_tensor("alpha", [P, CG], f32, kind="ExternalOutput")
    OP = mybir.AluOpType

    gcol = np.concatenate([[0], np.cumsum(GBG * cntG)]).astype(np.int64)

    with tile.TileContext(nc) as tc:
        with (
            tc.tile_pool(name="stage", bufs=1) as spool,
            tc.tile_pool(name="ttmp", bufs=2) as tpool,
            tc.tile_pool(name="etmp", bufs=2) as epool,
            tc.tile_pool(name="wbc", bufs=2) as wbpool,
            tc.tile_pool(name="qld", bufs=2) as qpool,
            tc.tile_pool(name="work", bufs=2) as wpool,
            tc.tile_pool(name="alph", bufs=2) as apool,
        ):
            # exp(e) in G layout
            e_sb = spool.tile([P, CG], f32)
            expe16 = spool.tile([P, CG], f16)
            nc.sync.dma_start(out=e_sb[:], in_=d_e[:])
            STRIP = 2048
            for s0 in range(0, CG, STRIP):
                s1 = min(s0 + STRIP, CG)
                nc.scalar.activation(expe16[:, s0:s1], e_sb[:, s0:s1],
                                     mybir.ActivationFunctionType.Exp)

            # 8-way flat-table all-reduce on 128 partitions
            FT = NT // P
            Tacc = spool.tile([P, FT], f32)
            nc.sync.dma_start(out=Tacc[:], in_=d_Tall[:, 0, :])
            for c in range(1, N_CORES):
                tmp = tpool.tile([P, FT], f32, tag="tt")
                nc.sync.dma_start(out=tmp[:], in_=d_Tall[:, c, :])
                nc.vector.tensor_tensor(out=Tacc[:], in0=Tacc[:],
                                        in1=tmp[:], op=OP.add)
            nc.vector.tensor_scalar_add(out=Tacc[:], in0=Tacc[:],
                                        scalar1=1e-16)
            nc.vector.reciprocal(out=Tacc[:], in_=Tacc[:])
            W16 = spool.tile([P, FT], f16)
            nc.vector.tensor_scalar_min(out=W16[:], in0=Tacc[:],
                                        scalar1=60000.0)
            nc.sync.dma_start(out=d_WT[:], in_=W16[:])

            for g in range(NGG):
                k = int(cntG[g])
                ncols = GBG * k
                col = int(gcol[g])
                # this group's W slice: flat t range [g*4096, (g+1)*4096)
                # broadcast to all partitions - contiguous per-partition run
                Wbc = wbpool.tile([P, GBG * MQG], f16, tag="wbc")
                src = APC(d_WT[:].tensor, g * GBG * MQG,
                          [[0, P], [1, GBG * MQG]])
                nc.scalar.dma_start(out=Wbc[:], in_=src)

                Qg = qpool.tile([P, GBG * k, MQG], f16, tag="Qg")
                nc.sync.dma_start(out=Qg[:], in_=d_Q[:, col:col + ncols, :])
                Pg = wpool.tile([P, GBG * k, MQG], f16, tag="Pg")
                nc.vector.tensor_tensor(
                    out=Pg[:].rearrange("p (b k) q -> p b k q", b=GBG),
                    in0=Qg[:].rearrange("p (b k) q -> p b k q", b=GBG),
                    in1=Wbc[:].rearrange("p (b q) -> p b q", b=GBG)
                    [:, :, None, :].broadcast_to([P, GBG, k, MQG]),
                    op=OP.mult)
                # 3-level pairwise max tree over q (exact: one nonzero)
                m1 = wpool.tile([P, GBG * k, 4], f16, tag="m1")
                nc.vector.tensor_tensor(out=m1[:], in0=Pg[:, :, 0:4],
                                        in1=Pg[:, :, 4:8], op=OP.max)
                m2 = wpool.tile([P, GBG * k, 2], f16, tag="m2")
                nc.vector.tensor_tensor(out=m2[:], in0=m1[:, :, 0:2],
                                        in1=m1[:, :, 2:4], op=OP.max)
                wg = wpool.tile([P, GBG * k], f16, tag="wg")
                nc.vector.tensor_tensor(out=wg[:], in0=m2[:, :, 0],
                                        in1=m2[:, :, 1], op=OP.max)
                alph = apool.tile([P, GBG * k], f32, tag="al")
                nc.vector.tensor_tensor(out=alph[:, 0:ncols],
                                        in0=expe16[:, col:col + ncols],
                                        in1=wg[:, 0:ncols], op=OP.mult)
                nc.scalar.dma_start(out=d_alpha[:, col:col + ncols],
                                  in_=alph[:, 0:ncols])
    nc.compile()
    return nc


def _get_neffs(meta):
    key = (meta["CS"], meta["cntS"].tobytes(),
           meta["CG"], meta["cntG"].tobytes())
    if key not in _cache:
        _cache[key] = (_build_s(meta["cntS"], meta["CS"]),
                       _build_g(meta["cntG"], meta["CG"]))
    return _cache[key]


def _layout(t, mq, nb, group):
    """Common bucket-sorted layout. Returns per-group cnt, per-bucket
    capacities, total columns C, and per-core (order, part, col)."""
    r_all = (t // mq).astype(np.int32).reshape(N_CORES, EC)
    counts = np.empty((N_CORES, nb), np.int64)
    for c in range(N_CORES):
        counts[c] = np.bincount(r_all[c], minlength=nb)
    need = -(-counts.max(axis=0) // P)            # ceil over max core count
    ngrp = nb // group
    cnt = np.maximum(need.reshape(ngrp, group).max(axis=1), 1).astype(
        np.int32)
    percnt = np.repeat(cnt, group)                # per-bucket capacity
    C = int(percnt.sum())
    base = np.concatenate([[0], np.cumsum(percnt)])[:-1]
    placements = []
    for c in range(N_CORES):
        r = r_all[c]
        order = np.argsort(r, kind="stable")
        rs = r[order]
        starts = np.concatenate([[0], np.cumsum(counts[c])])
        rank = np.arange(EC, dtype=np.int64) - starts[rs]
        part = (rank % P).astype(np.int32)
        col = (base[rs] + rank // P).astype(np.int32)
        placements.append((order, part, col))
    return cnt, percnt, C, placements


def prep_inputs(e, edge_index):
    e = np.asarray(e, dtype=np.float32).reshape(-1)
    t = np.asarray(edge_index)[1].astype(np.int64)
    e_all = e.reshape(N_CORES, EC)
    qS_all = (t % MQS).astype(np.int32).reshape(N_CORES, EC)
    qG_all = (t % MQG).astype(np.int32).reshape(N_CORES, EC)

    cntS_g, percntS, CS, placeS = _layout(t, MQS, NBS, 1)
    cntG, percntG, CG, placeG = _layout(t, MQG, NBG, GBG)

    iota = np.arange(MQS, dtype=np.float16)[None, :].repeat(P, axis=0)
    ident = np.eye(MQS, dtype=np.float32)

    in_maps_s, in_maps_g = [], []
    for c in range(N_CORES):
        orderS, partS, colS = placeS[c]
        e_padS = np.full((P, CS), -100.0, np.float32)
        q16 = np.zeros((P, CS), np.float16)
        e_padS[partS, colS] = e_all[c][orderS]
        q16[partS, colS] = qS_all[c][orderS].astype(np.float16)
        in_maps_s.append({"e": e_padS, "q": q16, "iota": iota,
                          "ident": ident})

        orderG, partG, colG = placeG[c]
        e_padG = np.full((P, CG), -100.0, np.float32)
        Q8 = np.zeros((P, CG, MQG), np.float16)
        e_padG[partG, colG] = e_all[c][orderG]
        Q8[partG, colG, qG_all[c][orderG]] = 1.0
        in_maps_g.append({"e": e_padG, "Q": Q8, "Tall": None})

    meta = {"CS": CS, "cntS": percntS.astype(np.int32),
            "CG": CG, "cntG": cntG, "placeG": placeG,
            "in_maps_g": in_maps_g}
    return in_maps_s, meta


def make_g_maps(res_s, meta):
    Tall = np.stack([res_s.results[c]["T"].reshape(P, NT // P)
                     for c in range(N_CORES)], axis=1)
    for c in range(N_CORES):
        meta["in_maps_g"][c]["Tall"] = Tall
    return meta["in_maps_g"]


def unpack_alpha(res_g, meta):
    alpha = np.empty(NUM_EDGES, dtype=np.float32)
    for c in range(N_CORES):
        order, part, col = meta["placeG"][c]
        a = res_g.results[c]["alpha"]
        shard = np.empty(EC, dtype=np.float32)
        shard[order] = a[part, col]
        alpha[c * EC:(c + 1) * EC] = shard
    return alpha


def kernel(e, edge_index, num_nodes):
    assert int(num_nodes) == NUM_NODES
    in_maps_s, meta = prep_inputs(e, edge_index)
    nc_s, nc_g = _get_neffs(meta)
    res_s = bass_utils.run_bass_kernel_spmd(nc_s, in_maps_s,
                                            core_ids=list(range(N_CORES)))
    in_maps_g = make_g_maps(res_s, meta)
    res_g = bass_utils.run_bass_kernel_spmd(nc_g, in_maps_g,
                                            core_ids=list(range(N_CORES)))
    return unpack_alpha(res_g, meta)
